# revision 22
# baseline (speedup 1.0000x reference)
"""GAT (graph attention) forward on 8 TRN2 NeuronCores, Bass/Tile.

Sharding: target nodes (rows of the output) split into 8 blocks of 512.
Each core redundantly computes the projected features h for ALL nodes
(cheap: one K=128 matmul chain) and then its own 512-row slice of the
attention + aggregation + skip + ELU.  No collectives.

Score factorization trick: with z[m,n] = s_tgt[m] + s_src[n] and
leaky(z) = max(z, 0.2 z),

    exp(leaky(z)) = max(e^z, e^{0.2 z})
                  = e^{0.2 s_src[n]} * max(u_m * w_n, p_m)

where u = e^{s_tgt}, p = e^{0.2 s_tgt}, w = e^{0.8 s_src}.  The leading
per-target factor cancels in the softmax normalization, so the kernel
computes only  et[m,n] = M01[m,n] * max(u_m * w_n, p_m)  per tile:
one DVE tensor_scalar with two per-partition scalars (mult then max)
and one DVE tensor_tensor multiply with the 0/1 bf16 mask, both bf16
(2x DVE mode).  The aggregation matmul runs in bf16 with a ones-column
appended to h so the softmax denominator falls out of the same matmul.

Projection/skip/score matmuls run as float32r (full-rate PE, tf32-ish).
The mask is transposed and converted to multiplicative 0/1 bf16 on the
host so the device only ever does contiguous row DMA.

Input DMA goes through the SP engine's hardware DGE queue (qSPDynamicHW)
which starts transfers ~8us earlier than the SWDGE ring and costs no
gpsimd dispatch; only the output DMA uses gpsimd.  The S3_LW
(weight-load) instruction can carry only one semaphore wait, so
_split_multi_waits rewrites any instruction Tile scheduled with 2+.
"""

import numpy as np
from contextlib import ExitStack

import concourse.bass as bass
import concourse.mybir as mybir
from concourse.tile import TileContext
from concourse.masks import make_identity
from concourse.bass_utils import run_bass_kernel_spmd

F32 = mybir.dt.float32
F32R = mybir.dt.float32r
BF16 = mybir.dt.bfloat16
AF = mybir.ActivationFunctionType
OP = mybir.AluOpType

N, FIN, H, FOUT = 4096, 128, 4, 64
G = H * FOUT
NCORES = 8
NLOC = N // NCORES          # local target rows per core
NCH = N // 128              # source (m) chunks
LCH = NLOC // 128           # local output row chunks
HE = FOUT + 1               # h_ext columns (ones col at index FOUT)
XPC = 8                     # phase-1 chunks per xT DMA piece
NXP = NCH // XPC            # xT DMA pieces


def build_program():
    # cpack: xT | wproj | wtgt | wsrc | wskip | brow | xTloc   (fp32 bits)
    cw = N + G + H + H + G + G + NLOC

    nc = bass.Bass()
    d_cpack = nc.declare_dram_parameter("cpack", [128, cw], F32R, isOutput=False)
    d_mask = nc.declare_dram_parameter("mask01", [N, NLOC], BF16, isOutput=False)
    d_sel = nc.declare_dram_parameter("selc", [4, 4 * 128], BF16, isOutput=False)
    d_out = nc.declare_dram_parameter("out", [NLOC, G], F32, isOutput=True)

    with TileContext(nc) as tc, ExitStack() as ctx:
        cp = ctx.enter_context(tc.tile_pool(name="const", bufs=1))
        sb_cpack = cp.tile([128, cw], F32R, tag="cpack")
        o = 0
        xTr = sb_cpack[:, o:o + N]; o += N
        wpsr = sb_cpack[:, o:o + G + H]; o += G + H      # wproj | wtgt merged
        wsrcr = sb_cpack[:, o:o + H]; o += H
        wskipr = sb_cpack[:, o:o + G]; o += G
        browr = sb_cpack[:, o:o + G]; o += G             # bias in row 0
        xTlocr = sb_cpack[:, o:o + NLOC]; o += NLOC

        sb_sel = cp.tile([4, 4 * 128], BF16, tag="sel")   # one-hot head rows
        sb_id = cp.tile([128, 128], F32, tag="ident")
        sb_h = cp.tile([128, NCH * H * HE], BF16, tag="hext")
        sb_w = cp.tile([128, H * NLOC], BF16, tag="wbc")      # e^{0.8 s_src}
        sb_wrow = cp.tile([4, NLOC], BF16, tag="wrow")
        sb_ones1 = cp.tile([1, 128], F32R, tag="ones1")
        sb_u = cp.tile([128, NCH * H], F32, tag="uexp")       # e^{s_tgt}
        sb_p = cp.tile([128, NCH * H], F32, tag="pexp")       # e^{0.2 s_tgt}
        sb_m = [cp.tile([128, NLOC], BF16, tag=f"m{j}", name=f"m{j}")
                for j in range(NCH)]

        # ---- DMA (SP HWDGE): weights, then xT pieces interleaved w/ mask --
        def dma_mask(j):
            nc.sync.dma_start(out=sb_m[j][:],
                              in_=d_mask[j * 128:(j + 1) * 128, :])

        nc.sync.dma_start(out=sb_sel[:], in_=d_sel[:])
        nc.sync.dma_start(out=sb_cpack[:, N:cw], in_=d_cpack[:, N:cw])
        mj = 0
        for p in range(NXP):
            w0 = p * XPC * 128
            nc.sync.dma_start(out=sb_cpack[:, w0:w0 + XPC * 128],
                              in_=d_cpack[:, w0:w0 + XPC * 128])
            for _ in range(4 if p else 2):
                dma_mask(mj); mj += 1
        while mj < NCH:
            dma_mask(mj); mj += 1

        make_identity(nc, sb_id[:])
        nc.vector.memset(sb_ones1[:].bitcast(F32), 1.0)

        # h_ext view: [128, c, HE]; chunk (j, head) at index j*H + head
        hv = sb_h[:].rearrange("p (c w) -> p c w", w=HE)
        nc.vector.memset(hv[:, :, FOUT:FOUT + 1], 1.0)

        # ---- phase 0: b = s_src(local), w = e^{0.8 b} broadcast -----------
        with tc.tile_pool(name="ps0", bufs=1, space="PSUM") as ps0:
            pb = ps0.tile([4, NLOC], F32, tag="pb")
            nc.tensor.matmul(pb[:], wsrcr, xTlocr, start=True, stop=True)
            nc.scalar.activation(sb_wrow[:], pb[:], AF.Exp, scale=0.8)
            pwb = ps0.tile([128, NLOC], F32, tag="pwb")
            for hh in range(H):
                nc.tensor.matmul(pwb[:], sb_sel[0:4, hh * 128:(hh + 1) * 128],
                                 sb_wrow[0:4, :], start=True, stop=True)
                nc.scalar.copy(sb_w[:, hh * NLOC:(hh + 1) * NLOC], pwb[:])

        # ---- phase 1: h = x @ proj (all heads) + s_tgt, u/p = exp ---------
        # u/p exps on scalar; the bulk h copy on gpsimd so u-production
        # (which paces the hot loop) is not stuck behind 357ns copies.
        with tc.tile_pool(name="ps1", bufs=3, space="PSUM") as ps1:
            for j in range(NCH):
                ph = ps1.tile([128, G + H], F32, tag="ph")
                lhs = xTr[:, j * 128:(j + 1) * 128]
                nc.tensor.matmul(ph[:], lhs, wpsr, start=True, stop=True)
                src_h = ph[:, 0:G].rearrange("p (hh f) -> p hh f", f=FOUT)
                nc.scalar.activation(sb_u[:, j * H:(j + 1) * H],
                                     ph[:, G:G + H], AF.Exp)
                nc.scalar.activation(sb_p[:, j * H:(j + 1) * H],
                                     ph[:, G:G + H], AF.Exp, scale=0.2)
                nc.scalar.copy(hv[:, j * H:(j + 1) * H, 0:FOUT], src_h)

        # ---- phase 3: attention main loop ---------------------------------
        po = []
        pso = ctx.enter_context(tc.tile_pool(name="pso", bufs=1, space="PSUM"))
        for hh in range(H):
            po.append(pso.tile([HE, NLOC], F32, tag=f"po{hh}", name=f"po{hh}"))

        # af tiles assembled per (li, head-block) as heads complete; the
        # skip+bias+ELU combine runs per li once its head-3 block is in.
        fp = ctx.enter_context(tc.tile_pool(name="fin", bufs=1))
        afs = [fp.tile([128, G], F32, tag=f"af{li}", name=f"af{li}")
               for li in range(LCH)]

        def head_finalize(hh, pon_hh, psf, fp2):
            for li in range(LCH):
                pt = psf.tile([128, HE], F32, tag="pt")
                nc.tensor.transpose(pt[0:128, 0:HE],
                                    pon_hh[:, li * 128:(li + 1) * 128],
                                    sb_id[0:HE, 0:HE])
                rcp = fp2.tile([128, 1], F32, tag="rcp")
                nc.vector.reciprocal(rcp[:], pt[:, FOUT:FOUT + 1])
                nc.vector.tensor_scalar(afs[li][:, hh * FOUT:(hh + 1) * FOUT],
                                        pt[:, 0:FOUT], rcp[:], None, OP.mult)

        pon = []
        with tc.tile_pool(name="work", bufs=8) as wp, \
             tc.tile_pool(name="fin2", bufs=2) as fp2, \
             tc.tile_pool(name="psf", bufs=2, space="PSUM") as psf:
            for hh in range(H):
                wb = sb_w[:, hh * NLOC:(hh + 1) * NLOC]
                for j in range(NCH):
                    c = j * H + hh
                    t1 = wp.tile([128, NLOC], BF16, tag="t1")
                    nc.vector.tensor_scalar(t1[:], wb, sb_u[:, c:c + 1],
                                            sb_p[:, c:c + 1], OP.mult, OP.max)
                    if hh > 0 and j % 2 == 1:
                        et = wp.tile([128, NLOC], BF16, tag="etg")
                        nc.gpsimd.tensor_tensor(et[:], t1[:], sb_m[j][:], OP.mult)
                    else:
                        et = wp.tile([128, NLOC], BF16, tag="et")
                        nc.vector.tensor_tensor(et[:], t1[:], sb_m[j][:], OP.mult)
                    nc.tensor.matmul(po[hh][:], hv[:, c, :], et[:],
                                     start=(j == 0), stop=(j == NCH - 1))
                # copy this head's accumulator out of PSUM so the PE can
                # transpose from SBUF, then fold this head's columns into af
                # (all overlapped with the next head's hot loop)
                pos = cp.tile([HE, NLOC], F32, tag=f"pos{hh}", name=f"pos{hh}")
                nc.scalar.copy(pos[:], po[hh][:])
                pon.append(pos)
                head_finalize(hh, pos, psf, fp2)

            # ---- tail: skip+bias, ELU, store (per li, pipelined) ----------
            for li in range(LCH):
                af = afs[li]
                pskip = psf.tile([128, G], F32, tag="pskip")
                nc.tensor.matmul(pskip[:], xTlocr[:, li * 128:(li + 1) * 128],
                                 wskipr, start=True, stop=False,
                                 skip_group_check=True)
                nc.tensor.matmul(pskip[:], sb_ones1[:], browr[0:1, :],
                                 start=False, stop=True, skip_group_check=True)
                nc.vector.tensor_tensor(af[:], af[:], pskip[:], OP.add)
                # ELU(z) = max(z,0) + exp(min(z,0)) - 1
                mn = fp2.tile([128, G], F32, tag="mn")
                nc.vector.tensor_scalar(mn[:], af[:], 0.0, None, OP.min)
                ex = fp2.tile([128, G], F32, tag="ex")
                nc.scalar.activation(ex[:], mn[:], AF.Exp)
                nc.vector.tensor_scalar(af[:], af[:], 0.0, None, OP.max)
                nc.vector.tensor_tensor(af[:], af[:], ex[:], OP.add)
                nc.vector.tensor_scalar(af[:], af[:], -1.0, None, OP.add)
                nc.gpsimd.dma_start(out=d_out[li * 128:(li + 1) * 128, :], in_=af[:])

    _split_multi_waits(nc)
    return nc


def _split_multi_waits(nc):
    """walrus on this toolchain allows only one semaphore-wait command on
    most compute-engine instructions (S3_LW / S3D3_* structs).  Tile's
    scheduler freely emits 2+.  Move all but one wait onto an injected
    same-engine NoOp right before the offending instruction."""
    skip = (mybir.InstEventSemaphore,)
    k = 0
    for f in nc.m.functions:
        for blk in f.blocks:
            new = []
            for ins in blk.instructions:
                si = getattr(ins, "sync_info", None)
                w = list(si.on_wait) if si is not None and si.on_wait else []
                if len(w) > 1 and not isinstance(ins, skip):
                    for wx in w[:-1]:
                        nop = mybir.InstNoOp(name=f"waitsplit-{k}", ins=[], outs=[])
                        nop.engine = ins.engine
                        nop.sync_info = mybir.SyncInfo(on_wait=[wx], on_update=[])
                        new.append(nop)
                        k += 1
                    ins.sync_info = mybir.SyncInfo(on_wait=w[-1:],
                                                   on_update=list(si.on_update))
                new.append(ins)
            blk.instructions[:] = new


_PROG = None


def _get_prog():
    global _PROG
    if _PROG is None:
        _PROG = build_program()
    return _PROG


def make_in_maps(x, mask, proj_param, score_src, score_tgt, skip_w, bias):
    import ml_dtypes
    x = np.asarray(x, np.float32)
    mask = np.asarray(mask, np.float32)
    proj = np.asarray(proj_param, np.float32)
    a_src = np.asarray(score_src, np.float32)[:, :, 0]       # [H, FOUT]
    a_tgt = np.asarray(score_tgt, np.float32)[:, :, 0]
    skip = np.asarray(skip_w, np.float32)
    b = np.asarray(bias, np.float32)

    xT = np.ascontiguousarray(x.T)                           # [128, N]
    wproj = np.ascontiguousarray(proj.transpose(1, 0, 2).reshape(FIN, G))
    w_src = np.einsum('hif,hf->ih', proj, a_src)             # [FIN, H]
    w_tgt = np.einsum('hif,hf->ih', proj, a_tgt)
    wskip = np.ascontiguousarray(skip.T)                     # [128, G]
    brow = np.zeros((128, G), np.float32)
    brow[0, :] = b
    mask01 = (mask == 0.0).astype(ml_dtypes.bfloat16)        # [N, N]

    sel = np.zeros((4, 4 * 128), ml_dtypes.bfloat16)
    for hh in range(H):
        sel[hh, hh * 128:(hh + 1) * 128] = 1

    in_maps = []
    for c in range(NCORES):
        r0 = c * NLOC
        cpack = np.ascontiguousarray(np.concatenate(
            [xT, wproj, w_tgt, w_src, wskip, brow, xT[:, r0:r0 + NLOC]],
            axis=1), np.float32)
        in_maps.append({
            "cpack": cpack,
            "mask01": np.ascontiguousarray(mask01[r0:r0 + NLOC, :].T),
            "selc": sel,
        })
    return in_maps


def run(in_maps, trace=False, **kw):
    res = run_bass_kernel_spmd(_get_prog(), in_maps, list(range(NCORES)),
                               trace=trace, **kw)
    out = np.concatenate([res.results[c]["out"] for c in range(NCORES)], axis=0)
    return out, res


def kernel(x, mask, proj_param, score_src, score_tgt, skip_w, bias):
    in_maps = make_in_maps(x, mask, proj_param, score_src, score_tgt, skip_w, bias)
    out, _ = run(in_maps)
    return out.astype(np.float32)


# revision 25
# speedup vs baseline: 1.3254x; 1.3254x over previous
"""GAT (graph attention) forward on 8 TRN2 NeuronCores, Bass/Tile.

Sharding: target nodes (rows of the output) split into 8 blocks of 512.
Each core redundantly computes the projected features h for ALL nodes
(cheap: one K=128 matmul chain) and then its own 512-row slice of the
attention + aggregation + skip + ELU.  No collectives.

Score factorization trick: with z[m,n] = s_tgt[m] + s_src[n] and
leaky(z) = max(z, 0.2 z),

    exp(leaky(z)) = max(e^z, e^{0.2 z})
                  = e^{0.2 s_src[n]} * max(u_m * w_n, p_m)

where u = e^{s_tgt}, p = e^{0.2 s_tgt}, w = e^{0.8 s_src}.  The leading
per-target factor cancels in the softmax normalization, so the kernel
computes only  et[m,n] = M01[m,n] * max(u_m * w_n, p_m)  per tile:
one DVE tensor_scalar with two per-partition scalars (mult then max)
and one DVE tensor_tensor multiply with the 0/1 bf16 mask, both bf16
(2x DVE mode).  The aggregation matmul runs in bf16 with a ones-column
appended to h so the softmax denominator falls out of the same matmul.

Projection/skip/score matmuls run as float32r (full-rate PE, tf32-ish).
The mask is transposed and converted to multiplicative 0/1 bf16 on the
host so the device only ever does contiguous row DMA.

Input DMA goes through the SP engine's hardware DGE queue (qSPDynamicHW)
which starts transfers ~8us earlier than the SWDGE ring and costs no
gpsimd dispatch; only the output DMA uses gpsimd.  The S3_LW
(weight-load) instruction can carry only one semaphore wait, so
_split_multi_waits rewrites any instruction Tile scheduled with 2+.
"""

import numpy as np
from contextlib import ExitStack

import concourse.bass as bass
import concourse.mybir as mybir
from concourse.tile import TileContext
from concourse.masks import make_identity
from concourse.bass_utils import run_bass_kernel_spmd

F32 = mybir.dt.float32
F32R = mybir.dt.float32r
BF16 = mybir.dt.bfloat16
AF = mybir.ActivationFunctionType
OP = mybir.AluOpType

N, FIN, H, FOUT = 4096, 128, 4, 64
G = H * FOUT
NCORES = 8
NLOC = N // NCORES          # local target rows per core
NCH = N // 128              # source (m) chunks
LCH = NLOC // 128           # local output row chunks
HE = FOUT + 1               # h_ext columns (ones col at index FOUT)
XPC = 8                     # phase-1 chunks per xT DMA piece
NXP = NCH // XPC            # xT DMA pieces


def build_program():
    # cpack: xT | wproj | wtgt | wsrc | wskip | brow | xTloc   (fp32 bits)
    cw = N + G + H + H + G + G + NLOC

    nc = bass.Bass()
    d_cpack = nc.declare_dram_parameter("cpack", [128, cw], F32R, isOutput=False)
    d_mask = nc.declare_dram_parameter("mask01", [N, NLOC], BF16, isOutput=False)
    d_sel = nc.declare_dram_parameter("selc", [4, 4 * 128], BF16, isOutput=False)
    d_out = nc.declare_dram_parameter("out", [NLOC, G], F32, isOutput=True)

    with TileContext(nc) as tc, ExitStack() as ctx:
        cp = ctx.enter_context(tc.tile_pool(name="const", bufs=1))
        sb_cpack = cp.tile([128, cw], F32R, tag="cpack")
        o = 0
        xTr = sb_cpack[:, o:o + N]; o += N
        wpsr = sb_cpack[:, o:o + G + H]; o += G + H      # wproj | wtgt merged
        wsrcr = sb_cpack[:, o:o + H]; o += H
        wskipr = sb_cpack[:, o:o + G]; o += G
        browr = sb_cpack[:, o:o + G]; o += G             # bias in row 0
        xTlocr = sb_cpack[:, o:o + NLOC]; o += NLOC

        sb_sel = cp.tile([4, 4 * 128], BF16, tag="sel")   # one-hot head rows
        sb_id = cp.tile([128, 128], F32, tag="ident")
        sb_h = cp.tile([128, NCH * H * HE], BF16, tag="hext")
        sb_w = cp.tile([128, H * NLOC], BF16, tag="wbc")      # e^{0.8 s_src}
        sb_wrow = cp.tile([4, NLOC], BF16, tag="wrow")
        sb_ones1 = cp.tile([1, 128], F32R, tag="ones1")
        sb_u = cp.tile([128, NCH * H], F32, tag="uexp")       # e^{s_tgt}
        sb_p = cp.tile([128, NCH * H], F32, tag="pexp")       # e^{0.2 s_tgt}
        # mask chunk pairs: sb_m[k] holds source chunks 2k and 2k+1
        sb_m = [cp.tile([128, 2 * NLOC], BF16, tag=f"m{k}", name=f"m{k}")
                for k in range(NCH // 2)]

        # ---- DMA (SP HWDGE): weights, then xT pieces interleaved w/ mask --
        def dma_mask(k):
            mv = sb_m[k][:].rearrange("p (c n) -> p c n", n=NLOC)
            dv = d_mask[k * 256:(k + 1) * 256, :].rearrange(
                "(c p) n -> p c n", p=128)
            nc.sync.dma_start(out=mv, in_=dv)

        nc.sync.dma_start(out=sb_sel[:], in_=d_sel[:])
        nc.sync.dma_start(out=sb_cpack[:, N:cw], in_=d_cpack[:, N:cw])
        mj = 0
        for p in range(NXP):
            w0 = p * XPC * 128
            nc.sync.dma_start(out=sb_cpack[:, w0:w0 + XPC * 128],
                              in_=d_cpack[:, w0:w0 + XPC * 128])
            for _ in range(2 if p else 1):
                dma_mask(mj); mj += 1
        while mj < NCH // 2:
            dma_mask(mj); mj += 1

        make_identity(nc, sb_id[:])
        nc.vector.memset(sb_ones1[:].bitcast(F32), 1.0)

        # h_ext view: [128, c, HE]; chunk (j, head) at index j*H + head
        hv = sb_h[:].rearrange("p (c w) -> p c w", w=HE)
        nc.vector.memset(hv[:, :, FOUT:FOUT + 1], 1.0)

        # ---- phase 0: b = s_src(local), w = e^{0.8 b} broadcast -----------
        with tc.tile_pool(name="ps0", bufs=1, space="PSUM") as ps0:
            pb = ps0.tile([4, NLOC], F32, tag="pb")
            nc.tensor.matmul(pb[:], wsrcr, xTlocr, start=True, stop=True)
            nc.scalar.activation(sb_wrow[:], pb[:], AF.Exp, scale=0.8)
            pwb = ps0.tile([128, NLOC], F32, tag="pwb")
            for hh in range(H):
                nc.tensor.matmul(pwb[:], sb_sel[0:4, hh * 128:(hh + 1) * 128],
                                 sb_wrow[0:4, :], start=True, stop=True)
                nc.vector.tensor_copy(sb_w[:, hh * NLOC:(hh + 1) * NLOC], pwb[:])

        # ---- phase 1: h = x @ proj (all heads) + s_tgt, u/p = exp ---------
        # u/p exps on scalar; the bulk h copy on gpsimd so u-production
        # (which paces the hot loop) is not stuck behind 357ns copies.
        with tc.tile_pool(name="ps1", bufs=3, space="PSUM") as ps1:
            for j in range(NCH):
                ph = ps1.tile([128, G + H], F32, tag="ph")
                lhs = xTr[:, j * 128:(j + 1) * 128]
                nc.tensor.matmul(ph[:], lhs, wpsr, start=True, stop=True)
                src_h = ph[:, 0:G].rearrange("p (hh f) -> p hh f", f=FOUT)
                nc.scalar.activation(sb_u[:, j * H:(j + 1) * H],
                                     ph[:, G:G + H], AF.Exp)
                nc.scalar.activation(sb_p[:, j * H:(j + 1) * H],
                                     ph[:, G:G + H], AF.Exp, scale=0.2)
                nc.scalar.copy(hv[:, j * H:(j + 1) * H, 0:FOUT], src_h)

        # ---- phase 3: attention main loop ---------------------------------
        po = []
        pso = ctx.enter_context(tc.tile_pool(name="pso", bufs=1, space="PSUM"))
        for hh in range(H):
            po.append(pso.tile([HE, NLOC], F32, tag=f"po{hh}", name=f"po{hh}"))

        # af tiles assembled per (li, head-block) as heads complete; the
        # skip+bias+ELU combine runs per li once its head-3 block is in.
        fp = ctx.enter_context(tc.tile_pool(name="fin", bufs=1))
        afs = [fp.tile([128, G], F32, tag=f"af{li}", name=f"af{li}")
               for li in range(LCH)]

        def head_finalize(hh, pon_hh, psf, fp2):
            for li in range(LCH):
                pt = psf.tile([128, HE], F32, tag="pt")
                nc.tensor.transpose(pt[0:128, 0:HE],
                                    pon_hh[:, li * 128:(li + 1) * 128],
                                    sb_id[0:HE, 0:HE])
                rcp = fp2.tile([128, 1], F32, tag="rcp")
                nc.vector.reciprocal(rcp[:], pt[:, FOUT:FOUT + 1])
                nc.vector.tensor_scalar(afs[li][:, hh * FOUT:(hh + 1) * FOUT],
                                        pt[:, 0:FOUT], rcp[:], None, OP.mult)

        pon = []
        with tc.tile_pool(name="work", bufs=8) as wp, \
             tc.tile_pool(name="fin2", bufs=2) as fp2, \
             tc.tile_pool(name="psf", bufs=2, space="PSUM") as psf:
            for hh in range(H):
                wb = sb_w[:, hh * NLOC:(hh + 1) * NLOC]
                for k in range(NCH // 2):
                    c0 = 2 * k * H + hh
                    c1 = (2 * k + 1) * H + hh
                    # t1/et slabs cover source chunks 2k, 2k+1 side by side;
                    # the 1024-col TT amortizes DVE per-op overhead
                    t1 = wp.tile([128, 2 * NLOC], BF16, tag="t1")
                    nc.vector.tensor_scalar(t1[:, 0:NLOC], wb, sb_u[:, c0:c0 + 1],
                                            sb_p[:, c0:c0 + 1], OP.mult, OP.max)
                    nc.vector.tensor_scalar(t1[:, NLOC:], wb, sb_u[:, c1:c1 + 1],
                                            sb_p[:, c1:c1 + 1], OP.mult, OP.max)
                    if hh > 0 and k % 3 == 2:
                        et = wp.tile([128, 2 * NLOC], BF16, tag="etg")
                        nc.gpsimd.tensor_tensor(et[:], t1[:], sb_m[k][:], OP.mult)
                    else:
                        et = wp.tile([128, 2 * NLOC], BF16, tag="et")
                        nc.vector.tensor_tensor(et[:], t1[:], sb_m[k][:], OP.mult)
                    nc.tensor.matmul(po[hh][:], hv[:, c0, :], et[:, 0:NLOC],
                                     start=(k == 0), stop=False)
                    nc.tensor.matmul(po[hh][:], hv[:, c1, :], et[:, NLOC:],
                                     start=False, stop=(k == NCH // 2 - 1))
                # copy this head's accumulator out of PSUM so the PE can
                # transpose from SBUF, then fold this head's columns into af
                # (all overlapped with the next head's hot loop)
                pos = cp.tile([HE, NLOC], F32, tag=f"pos{hh}", name=f"pos{hh}")
                nc.scalar.copy(pos[:], po[hh][:])
                pon.append(pos)
                head_finalize(hh, pos, psf, fp2)

            # ---- tail: skip+bias, ELU, store (per li, pipelined) ----------
            for li in range(LCH):
                af = afs[li]
                pskip = psf.tile([128, G], F32, tag="pskip")
                nc.tensor.matmul(pskip[:], xTlocr[:, li * 128:(li + 1) * 128],
                                 wskipr, start=True, stop=False,
                                 skip_group_check=True)
                nc.tensor.matmul(pskip[:], sb_ones1[:], browr[0:1, :],
                                 start=False, stop=True, skip_group_check=True)
                nc.vector.tensor_tensor(af[:], af[:], pskip[:], OP.add)
                # ELU(z) = max(z,0) + exp(min(z,0)) - 1
                mn = fp2.tile([128, G], F32, tag="mn")
                nc.vector.tensor_scalar(mn[:], af[:], 0.0, None, OP.min)
                ex = fp2.tile([128, G], F32, tag="ex")
                nc.scalar.activation(ex[:], mn[:], AF.Exp)
                nc.vector.tensor_scalar(af[:], af[:], 0.0, None, OP.max)
                nc.vector.tensor_tensor(af[:], af[:], ex[:], OP.add)
                nc.vector.tensor_scalar(af[:], af[:], -1.0, None, OP.add)
                nc.gpsimd.dma_start(out=d_out[li * 128:(li + 1) * 128, :], in_=af[:])

    _split_multi_waits(nc)
    return nc


def _split_multi_waits(nc):
    """walrus on this toolchain allows only one semaphore-wait command on
    most compute-engine instructions (S3_LW / S3D3_* structs).  Tile's
    scheduler freely emits 2+.  Move all but one wait onto an injected
    same-engine NoOp right before the offending instruction."""
    skip = (mybir.InstEventSemaphore,)
    k = 0
    for f in nc.m.functions:
        for blk in f.blocks:
            new = []
            for ins in blk.instructions:
                si = getattr(ins, "sync_info", None)
                w = list(si.on_wait) if si is not None and si.on_wait else []
                if len(w) > 1 and not isinstance(ins, skip):
                    for wx in w[:-1]:
                        nop = mybir.InstNoOp(name=f"waitsplit-{k}", ins=[], outs=[])
                        nop.engine = ins.engine
                        nop.sync_info = mybir.SyncInfo(on_wait=[wx], on_update=[])
                        new.append(nop)
                        k += 1
                    ins.sync_info = mybir.SyncInfo(on_wait=w[-1:],
                                                   on_update=list(si.on_update))
                new.append(ins)
            blk.instructions[:] = new


_PROG = None


def _get_prog():
    global _PROG
    if _PROG is None:
        _PROG = build_program()
    return _PROG


def make_in_maps(x, mask, proj_param, score_src, score_tgt, skip_w, bias):
    import ml_dtypes
    x = np.asarray(x, np.float32)
    mask = np.asarray(mask, np.float32)
    proj = np.asarray(proj_param, np.float32)
    a_src = np.asarray(score_src, np.float32)[:, :, 0]       # [H, FOUT]
    a_tgt = np.asarray(score_tgt, np.float32)[:, :, 0]
    skip = np.asarray(skip_w, np.float32)
    b = np.asarray(bias, np.float32)

    xT = np.ascontiguousarray(x.T)                           # [128, N]
    wproj = np.ascontiguousarray(proj.transpose(1, 0, 2).reshape(FIN, G))
    w_src = np.einsum('hif,hf->ih', proj, a_src)             # [FIN, H]
    w_tgt = np.einsum('hif,hf->ih', proj, a_tgt)
    wskip = np.ascontiguousarray(skip.T)                     # [128, G]
    brow = np.zeros((128, G), np.float32)
    brow[0, :] = b
    mask01 = (mask == 0.0).astype(ml_dtypes.bfloat16)        # [N, N]

    sel = np.zeros((4, 4 * 128), ml_dtypes.bfloat16)
    for hh in range(H):
        sel[hh, hh * 128:(hh + 1) * 128] = 1

    in_maps = []
    for c in range(NCORES):
        r0 = c * NLOC
        cpack = np.ascontiguousarray(np.concatenate(
            [xT, wproj, w_tgt, w_src, wskip, brow, xT[:, r0:r0 + NLOC]],
            axis=1), np.float32)
        in_maps.append({
            "cpack": cpack,
            "mask01": np.ascontiguousarray(mask01[r0:r0 + NLOC, :].T),
            "selc": sel,
        })
    return in_maps


def run(in_maps, trace=False, **kw):
    res = run_bass_kernel_spmd(_get_prog(), in_maps, list(range(NCORES)),
                               trace=trace, **kw)
    out = np.concatenate([res.results[c]["out"] for c in range(NCORES)], axis=0)
    return out, res


def kernel(x, mask, proj_param, score_src, score_tgt, skip_w, bias):
    in_maps = make_in_maps(x, mask, proj_param, score_src, score_tgt, skip_w, bias)
    out, _ = run(in_maps)
    return out.astype(np.float32)
